# revision 15
# baseline (speedup 1.0000x reference)
"""Trainium2 Bass kernel for nn_CASAtt_MultiHead_v1 (CAS attention block).

Reference computation (per sample):
    qkv = 1x1 conv (qkv_w) -> q, k, v                        [512, 56, 56] each
    q <- SE(dwconv3x3(q, sq_w, sq_b))   (per-head squeeze-excite)
    k <- SE(dwconv3x3(k, sk_w, sk_b))
    out = proj(dwconv3x3(q + k, dwc_w, dwc_b) * v) + proj_b + x

Distribution: data-parallel over batch, 2 samples per NeuronCore x 8 cores.

Strategy (all PE matmuls in fp8e4m3 with DoubleRow pairing, f32 PSUM):
  * GEMMs: contraction K=512 done as 2 DoubleRow matmuls (K=256 each,
    channel-block pairs) per 128-out block -- 2x fewer PE column-cycles
    than bf16.
  * The SE scales s_q, s_k are computed WITHOUT materializing conv1:
    with zero padding, mean(dwconv3x3(q)) is a per-channel linear
    combination of 9 cheap stats (total sum via drain accum_out, 2 edge
    row sums, 2 edge col sums, 4 corners).  Host precomputes the [C, 9]
    coefficient table.
  * m = s_q*conv_sq(q) + s_k*conv_sk(k) is then computed directly as 9
    cross-branch DoubleRow pair matmuls: tap j of q paired with tap j of
    k (q/k live in one SBUF tile, pair stride = buffer pitch), with the
    diagonal tap weights scaled by s at runtime on DVE.  This fuses both
    conv1 branches AND the SE-scale merge into one accumulation.
  * dwc conv on m: 4 DoubleRow tap-pair matmuls + 1 single matmul.
  * fp8 weights are prescaled by WS=32 (values ~0.05 would be subnormal
    in e4m3); drains compensate with ACT scale=1/WS.
  * Residual: host ships xr = x + proj_b in f32; the proj drain is a
    single DVE STT (psum * 1/WS + xr).
Conv matmuls operate on zero-padded HPxWP buffers (WP=60) over full
padded rows; garbage in pad columns is never read.
"""

import numpy as np

DIM = 512
NH = 4
HD = 128
HD4 = 32
B, H_FULL, W = 16, 56, 56
N_CORES = 8
WS = 32.0

H = H_FULL
WP = W + 4  # 60: even row pitch (4B-aligned fp8 rows for 2x ACT/DVE modes)
HP = H + 2
PADN = HP * WP          # 3480
PITCH = PADN + 2        # per-branch pitch inside the qk pair buffer
TAPS = [(dy, dx) for dy in (-1, 0, 1) for dx in (-1, 0, 1)]
DWC_PAIRS = [(0, 1), (2, 3), (4, 5), (6, 7)]
DWC_SINGLE = 8


def default_cfg():
    return dict(
        b_local=B // N_CORES,
        rows_per_tile=8,
        repeat=1,
    )


def build_nc(cfg):
    """Build + compile the Bacc program for one core (SPMD across 8)."""
    import concourse.bass as bass
    import concourse.mybir as mybir
    import concourse.tile as tile
    import bass_rust
    from concourse import bacc
    from contextlib import ExitStack

    f32 = mybir.dt.float32
    bf16 = mybir.dt.bfloat16
    fp8 = mybir.dt.float8e4
    DR = mybir.MatmulPerfMode.DoubleRow

    BL = cfg['b_local']
    TH = cfg['rows_per_tile']
    NT = H // TH
    assert NT * TH == H
    TN = TH * W
    TPAD = TH * WP
    AF = mybir.ActivationFunctionType
    AL = mybir.AluOpType

    nc = bacc.Bacc("TRN2", target_bir_lowering=False, debug=False,
                   enable_asserts=False, num_devices=N_CORES)

    # ---------------- DRAM I/O ----------------
    x_d = nc.dram_tensor("x8", [BL, DIM, H, W], fp8, kind="ExternalInput").ap()
    xr_d = nc.dram_tensor("xr", [BL, DIM, H, W], f32, kind="ExternalInput").ap()
    out_d = nc.dram_tensor("out", [BL, DIM, H, W], f32, kind="ExternalOutput").ap()
    # GEMM weights: [gemm][oc][kc2][128, 2, 128] fp8 (prescaled by WS)
    wg_d = {g: nc.dram_tensor(f"w_{g}", [NH, 2, HD, 2, HD], fp8,
                              kind="ExternalInput").ap()
            for g in ("q", "k", "v", "p")}
    # m-conv diag pairs: [head][tap][128, 2, 128] fp8 (prescaled by WS)
    dgm_d = nc.dram_tensor("dgm", [NH, HD, 9, 2, HD], fp8,
                           kind="ExternalInput").ap()
    # dwc diag: pairs [head][4][128, 2, 128] + singles [head][128, 128]
    dgw_d = nc.dram_tensor("dgw", [NH, HD, 4, 2, HD], fp8,
                           kind="ExternalInput").ap()
    dgs_d = nc.dram_tensor("dgs", [NH, HD, HD], fp8,
                           kind="ExternalInput").ap()
    # pooling coef [2 branches][head][128, 9] f32 (includes /npix)
    coef_d = nc.dram_tensor("coef", [2, NH, HD, 9], f32,
                            kind="ExternalInput").ap()
    # biases
    b1_d = [nc.dram_tensor(n, [DIM, 1], f32, kind="ExternalInput").ap()
            for n in ("sq_b", "sk_b")]
    dwcb_d = nc.dram_tensor("dwc_b", [DIM, 1], f32, kind="ExternalInput").ap()
    # SE weights (f32); seb1 already includes w1 @ conv-bias fold
    sew1_d = [nc.dram_tensor(n, [NH, HD, HD4], f32, kind="ExternalInput").ap()
              for n in ("se_w1q", "se_w1k")]
    seb1_d = [nc.dram_tensor(n, [NH, HD4, 1], f32, kind="ExternalInput").ap()
              for n in ("se_b1q", "se_b1k")]
    sew2_d = [nc.dram_tensor(n, [NH, HD4, HD], f32, kind="ExternalInput").ap()
              for n in ("se_w2q", "se_w2k")]
    seb2_d = [nc.dram_tensor(n, [NH, HD, 1], f32, kind="ExternalInput").ap()
              for n in ("se_b2q", "se_b2k")]

    dbg = cfg.get('debug', False)
    if dbg:
        dbg_qk_d = nc.dram_tensor("dbg_qk", [NH, HD, 2 * PITCH], fp8,
                                  kind="ExternalOutput").ap()
        dbg_m_d = nc.dram_tensor("dbg_m", [NH, HD, PITCH], fp8,
                                 kind="ExternalOutput").ap()
        dbg_s_d = nc.dram_tensor("dbg_s", [2, NH, HD, 1], f32,
                                 kind="ExternalOutput").ap()
        dbg_st_d = nc.dram_tensor("dbg_st", [2, NH, HD, 9], f32,
                                  kind="ExternalOutput").ap()
        dbg_pool_d = nc.dram_tensor("dbg_pool", [2, NH, HD, 1], f32,
                                    kind="ExternalOutput").ap()
        dbg_o2_d = nc.dram_tensor("dbg_o2", [H // cfg['rows_per_tile'],
                                             HD, 4 * cfg['rows_per_tile'] * W],
                                  fp8, kind="ExternalOutput").ap()
        dbg_v_d = nc.dram_tensor("dbg_v", [H // cfg['rows_per_tile'], NH,
                                           HD, cfg['rows_per_tile'] * W],
                                 mybir.dt.bfloat16, kind="ExternalOutput").ap()
        dbg_c2_d = nc.dram_tensor("dbg_c2", [H // cfg['rows_per_tile'], NH,
                                             HD, cfg['rows_per_tile'] * W],
                                  mybir.dt.bfloat16, kind="ExternalOutput").ap()

    with tile.TileContext(nc) as tc, ExitStack() as ctx:
        const = ctx.enter_context(tc.tile_pool(name="const", bufs=1))
        big = ctx.enter_context(tc.tile_pool(name="big", bufs=1))
        sdg = ctx.enter_context(tc.tile_pool(name="sdg", bufs=2))
        xpool = ctx.enter_context(tc.tile_pool(name="xpool", bufs=2))
        vpool = ctx.enter_context(tc.tile_pool(name="vpool", bufs=2))
        o2pool = ctx.enter_context(tc.tile_pool(name="o2pool", bufs=2))
        otpool = ctx.enter_context(tc.tile_pool(name="otpool", bufs=2))
        xrpool = ctx.enter_context(tc.tile_pool(name="xrpool", bufs=2))
        statpool = ctx.enter_context(tc.tile_pool(name="statpool", bufs=2))
        mmpool = ctx.enter_context(tc.tile_pool(name="mmpool", bufs=5, space="PSUM"))
        sepool = ctx.enter_context(tc.tile_pool(name="sepool", bufs=2, space="PSUM"))

        # ---------- persistent SBUF ----------
        # q/k padded pair buffers (per sample): q at [1, 1+PADN), k at
        # [PITCH+1, ...).  Per-sample buffers so sample b+1's GEMM drains
        # can overlap sample b's m-conv.
        qk = [[big.tile([HD, 2 * PITCH], fp8, name=f"qk{b}_{c}")
               for c in range(NH)] for b in range(BL)]
        mb = [[big.tile([HD, PITCH], fp8, name=f"m{b}_{c}")
               for c in range(NH)] for b in range(BL)]
        for row in qk + mb:
            for t in row:
                nc.vector.memset(t, 0.0)

        def pad3(t, half):
            return t[:, half * PITCH + 1:half * PITCH + 1 + PADN].rearrange(
                "p (h w) -> p h w", w=WP)

        qk3 = [[[pad3(t, half) for half in range(2)] for t in qk[b]]
               for b in range(BL)]
        m3 = [[pad3(t, 0) for t in mb[b]] for b in range(BL)]

        # ---------- static weights ----------
        wg = {}
        for g in ("q", "k", "v", "p"):
            wg[g] = [[const.tile([HD, 2, HD], fp8, name=f"w{g}_{oc}_{k2}")
                      for k2 in range(2)] for oc in range(NH)]
            for oc in range(NH):
                for k2 in range(2):
                    nc.sync.dma_start(wg[g][oc][k2], wg_d[g][oc, k2])
        # raw m-conv diag pairs, one tile per head [128, 9, 2, 128]
        dgm = [const.tile([HD, 9, 2, HD], fp8, name=f"dgm{c}") for c in range(NH)]
        for c in range(NH):
            nc.sync.dma_start(dgm[c], dgm_d[c])
        dgw = [const.tile([HD, 4, 2, HD], fp8, name=f"dgw{c}") for c in range(NH)]
        dgs = [const.tile([HD, HD], fp8, name=f"dgs{c}") for c in range(NH)]
        for c in range(NH):
            nc.sync.dma_start(dgw[c], dgw_d[c])
            nc.sync.dma_start(dgs[c], dgs_d[c])
        coef = [[const.tile([HD, 9], f32, name=f"coef{br}_{c}") for c in range(NH)]
                for br in range(2)]
        for br in range(2):
            for c in range(NH):
                nc.sync.dma_start(coef[br][c], coef_d[br, c])
        bias1 = [[const.tile([HD, 1], f32, name=f"b1_{br}_{c}") for c in range(NH)]
                 for br in range(2)]
        dwcb = [const.tile([HD, 1], f32, name=f"dwcb{c}") for c in range(NH)]
        inv_ws = const.tile([HD, 1], f32, name="inv_ws")
        nc.vector.memset(inv_ws, 1.0 / WS)
        for c in range(NH):
            sl = slice(c * HD, (c + 1) * HD)
            for br in range(2):
                nc.sync.dma_start(bias1[br][c], b1_d[br][sl])
            nc.sync.dma_start(dwcb[c], dwcb_d[sl])
        sew1 = [[const.tile([HD, HD4], f32, name=f"sew1_{br}_{c}") for c in range(NH)]
                for br in range(2)]
        seb1 = [[const.tile([HD4, 1], f32, name=f"seb1_{br}_{c}") for c in range(NH)]
                for br in range(2)]
        sew2 = [[const.tile([HD4, HD], f32, name=f"sew2_{br}_{c}") for c in range(NH)]
                for br in range(2)]
        seb2 = [[const.tile([HD, 1], f32, name=f"seb2_{br}_{c}") for c in range(NH)]
                for br in range(2)]
        for br in range(2):
            for c in range(NH):
                nc.sync.dma_start(sew1[br][c], sew1_d[br][c])
                nc.sync.dma_start(seb1[br][c], seb1_d[br][c])
                nc.sync.dma_start(sew2[br][c], sew2_d[br][c])
                nc.sync.dma_start(seb2[br][c], seb2_d[br][c])

        def pair_rhs(tbuf, r0, j, pitch):
            """[128, 2, TPAD] AP: tap j over padded rows r0+1..r0+TH of both
            halves of a pair buffer (pair stride = pitch)."""
            dy, dx = TAPS[j]
            base = 1 + (r0 + 1) * WP + dy * WP + dx
            return bass_rust.AP(
                tensor=tbuf.tensor, offset=tbuf.offset + base,
                ap=[[tbuf.shape[1], HD], [pitch, 2], [1, TPAD]])

        def tap_rhs(tbuf, r0, ja, jb):
            """[128, 2, TPAD] AP on a single-pitch buffer: taps ja, jb."""
            dya, dxa = TAPS[ja]
            dyb, dxb = TAPS[jb]
            da = dya * WP + dxa
            db = dyb * WP + dxb
            base = 1 + (r0 + 1) * WP + da
            return bass_rust.AP(
                tensor=tbuf.tensor, offset=tbuf.offset + base,
                ap=[[tbuf.shape[1], HD], [db - da, 2], [1, TPAD]])

        def single_rhs(tbuf, r0, j):
            dy, dx = TAPS[j]
            base = 1 + (r0 + 1) * WP + dy * WP + dx
            return tbuf[:, base:base + TPAD]

        def emit_body(rep):
            sfx = f"_r{rep}" if cfg['repeat'] > 1 else ""
            s_scale = [[[None] * NH for _ in range(2)] for _ in range(BL)]
            sdiag = [None] * BL

            def phase_qk(b):
                # q,k GEMMs -> qk pair buffers (fp8), with accum_out stats
                stats_s = [[statpool.tile([HD, NT], f32, tag=f"ss{br}_{oc}",
                                          name=f"ss{b}_{br}_{oc}{sfx}")
                            for oc in range(NH)] for br in range(2)]
                for t in range(NT):
                    r0 = t * TH
                    xt = xpool.tile([HD, 4 * TN], fp8, tag="xt",
                                    name=f"xt_b{b}_{t}{sfx}")
                    for kc in range(NH):
                        nc.sync.dma_start(
                            xt[:, kc * TN:(kc + 1) * TN].rearrange(
                                "p (h w) -> p h w", w=W),
                            x_d[b, kc * HD:(kc + 1) * HD, r0:r0 + TH, :])
                    for br, g in enumerate(("q", "k")):
                        for oc in range(NH):
                            ps = mmpool.tile([HD, TN], f32, tag="mm",
                                             name=f"g{b}_{g}_{t}_{oc}{sfx}")
                            for k2 in range(2):
                                rhs = xt[:, k2 * 2 * TN:(k2 + 1) * 2 * TN].rearrange(
                                    "p (two n) -> p two n", two=2)
                                nc.tensor.matmul(ps, wg[g][oc][k2], rhs,
                                                 start=(k2 == 0), stop=(k2 == 1),
                                                 perf_mode=DR)
                            nc.scalar.activation(
                                qk3[b][oc][br][:, 1 + r0:1 + r0 + TH, 1:1 + W],
                                ps.rearrange("p (h w) -> p h w", w=W),
                                AF.Identity, scale=inv_ws,
                                accum_out=stats_s[br][oc][:, t:t + 1])
                return stats_s

            def phase_se(b, stats_s):
                # boundary-corrected pooling + SE -> s_scale
                for br in range(2):
                    for oc in range(NH):
                        buf = qk[b][oc]
                        # flat offset of interior pixel (0, 0): slop(1) + row
                        # pad(WP) + col pad(1)
                        off = buf.offset + br * PITCH + 1
                        st = statpool.tile([HD, 9], f32, tag="st9",
                                           name=f"st9_{b}_{br}_{oc}{sfx}")
                        # S
                        nc.vector.tensor_reduce(st[:, 0:1], stats_s[br][oc],
                                                mybir.AxisListType.X, AL.add)
                        # rows 0 and H-1 (padded rows 1, H): offsets 61, H*WP+1
                        rows = bass_rust.AP(
                            tensor=buf.tensor, offset=off + WP + 1,
                            ap=[[buf.shape[1], HD], [(H - 1) * WP, 2], [1, W]])
                        nc.vector.tensor_reduce(st[:, 1:3], rows,
                                                mybir.AxisListType.X, AL.add)
                        # cols 0 and W-1: offsets 61, 61+W-1; stride WP over H
                        cols = bass_rust.AP(
                            tensor=buf.tensor, offset=off + WP + 1,
                            ap=[[buf.shape[1], HD], [W - 1, 2], [WP, H]])
                        nc.vector.tensor_reduce(st[:, 3:5], cols,
                                                mybir.AxisListType.X, AL.add)
                        # 4 corners
                        corners = bass_rust.AP(
                            tensor=buf.tensor, offset=off + WP + 1,
                            ap=[[buf.shape[1], HD], [(H - 1) * WP, 2], [W - 1, 2]])
                        nc.vector.tensor_copy(
                            st[:, 5:9].rearrange("p (a b) -> p a b", a=2), corners)
                        # pooled = sum(st * coef)
                        prod = statpool.tile([HD, 9], f32, tag="pr9",
                                             name=f"pr9_{b}_{br}_{oc}{sfx}")
                        nc.vector.tensor_tensor(prod, st, coef[br][oc], AL.mult)
                        pooled = const.tile([HD, 1], f32, tag="pooled", bufs=4,
                                            name=f"pool{b}_{br}_{oc}{sfx}")
                        nc.vector.tensor_reduce(pooled, prod,
                                                mybir.AxisListType.X, AL.add)
                        ps1 = sepool.tile([HD4, 1], f32, tag="se",
                                          name=f"se1_{b}_{br}_{oc}{sfx}")
                        nc.tensor.matmul(ps1, sew1[br][oc], pooled,
                                         start=True, stop=True)
                        hvec = const.tile([HD4, 1], f32, tag="hvec", bufs=4,
                                          name=f"h{b}_{br}_{oc}{sfx}")
                        nc.scalar.activation(hvec, ps1, AF.Relu,
                                             bias=seb1[br][oc])
                        ps2 = sepool.tile([HD, 1], f32, tag="se",
                                          name=f"se2_{b}_{br}_{oc}{sfx}")
                        nc.tensor.matmul(ps2, sew2[br][oc], hvec,
                                         start=True, stop=True)
                        s_sb = const.tile([HD, 1], f32, tag="s_scale", bufs=16,
                                          name=f"s{b}_{br}_{oc}{sfx}")
                        nc.scalar.activation(s_sb, ps2, AF.Sigmoid,
                                             bias=seb2[br][oc])
                        s_scale[b][br][oc] = s_sb
                        if dbg and b == 0 and rep == 0:
                            nc.sync.dma_start(dbg_st_d[br, oc], st)
                            nc.sync.dma_start(dbg_pool_d[br, oc], pooled)
                            nc.sync.dma_start(dbg_s_d[br, oc], s_sb)

            def phase_scale_diag(b):
                # scaled diag pairs: sd[:, j, 0, :] = s_q * dgm[:, j, 0, :]
                sd = [sdg.tile([HD, 9, 2, HD], fp8, tag=f"sd{c}",
                               name=f"sd{c}_b{b}{sfx}") for c in range(NH)]
                for c in range(NH):
                    for br in range(2):
                        nc.vector.tensor_scalar(
                            sd[c][:, :, br, :], dgm[c][:, :, br, :],
                            s_scale[b][br][c], None, AL.mult)
                sdiag[b] = sd

            def phase_mconv(b):
                # m = s_q*conv_sq(q) + s_k*conv_sk(k) : 9 DR pair MMs/tile
                for oc in range(NH):
                    # mbias = s_q*sq_b + s_k*sk_b
                    mb_t = const.tile([HD, 1], f32, tag="mbias", bufs=4,
                                      name=f"mb{b}_{oc}{sfx}")
                    nc.vector.tensor_scalar(mb_t, bias1[1][oc],
                                            s_scale[b][1][oc], None, AL.mult)
                    nc.vector.scalar_tensor_tensor(
                        mb_t, bias1[0][oc], s_scale[b][0][oc], mb_t,
                        AL.mult, AL.add)
                    for t in range(NT):
                        r0 = t * TH
                        ps = mmpool.tile([HD, TPAD], f32, tag="mm",
                                         name=f"mc{b}_{t}_{oc}{sfx}")
                        for j in range(9):
                            nc.tensor.matmul(ps, sdiag[b][oc][:, j],
                                             pair_rhs(qk[b][oc], r0, j, PITCH),
                                             start=(j == 0), stop=(j == 8),
                                             perf_mode=DR)
                        nc.scalar.activation(
                            m3[b][oc][:, 1 + r0:1 + r0 + TH, 1:1 + W],
                            ps.rearrange("p (h w) -> p h w",
                                         w=WP)[:, :, 1:1 + W],
                            AF.Identity, bias=mb_t, scale=inv_ws)

            def phase_out(b):
                # dwc conv + v GEMM + o2 + proj + residual
                for t in range(NT):
                    r0 = t * TH
                    xt = xpool.tile([HD, 4 * TN], fp8, tag="xt",
                                    name=f"x2_b{b}_{t}{sfx}")
                    for kc in range(NH):
                        nc.sync.dma_start(
                            xt[:, kc * TN:(kc + 1) * TN].rearrange(
                                "p (h w) -> p h w", w=W),
                            x_d[b, kc * HD:(kc + 1) * HD, r0:r0 + TH, :])
                    o2 = o2pool.tile([HD, 4 * TN], fp8, tag="o2",
                                     name=f"o2_b{b}_{t}{sfx}")
                    for oc in range(NH):
                        # v = Wv x
                        psv = mmpool.tile([HD, TN], f32, tag="mm",
                                          name=f"v{b}_{t}_{oc}{sfx}")
                        for k2 in range(2):
                            rhs = xt[:, k2 * 2 * TN:(k2 + 1) * 2 * TN].rearrange(
                                "p (two n) -> p two n", two=2)
                            nc.tensor.matmul(psv, wg["v"][oc][k2], rhs,
                                             start=(k2 == 0), stop=(k2 == 1),
                                             perf_mode=DR)
                        vv = vpool.tile([HD, TN], bf16, tag=f"vt{oc}",
                                        name=f"vt{oc}_b{b}_{t}{sfx}")
                        nc.scalar.activation(vv, psv, AF.Identity, scale=inv_ws)
                        # c2 = dwc(m) + dwc_b
                        psc = mmpool.tile([HD, TPAD], f32, tag="mm",
                                          name=f"c2{b}_{t}_{oc}{sfx}")
                        for pi, (ja, jb) in enumerate(DWC_PAIRS):
                            nc.tensor.matmul(psc, dgw[oc][:, pi],
                                             tap_rhs(mb[b][oc], r0, ja, jb),
                                             start=(pi == 0), stop=False,
                                             perf_mode=DR)
                        nc.tensor.matmul(psc, dgs[oc],
                                         single_rhs(mb[b][oc], r0, DWC_SINGLE),
                                         start=False, stop=True)
                        c2t = o2pool.tile([HD, TN], bf16, tag="c2t", bufs=3,
                                          name=f"c2t_{oc}_b{b}_{t}{sfx}")
                        nc.scalar.activation(
                            c2t.rearrange("p (h w) -> p h w", w=W),
                            psc.rearrange("p (h w) -> p h w", w=WP)[:, :, 1:1 + W],
                            AF.Identity, bias=dwcb[oc], scale=inv_ws)
                        # o2 = c2 * v  (bf16 in, fp8 out)
                        nc.vector.tensor_tensor(
                            o2[:, oc * TN:(oc + 1) * TN], c2t, vv, AL.mult)
                        if dbg and b == 0 and rep == 0:
                            nc.sync.dma_start(dbg_v_d[t, oc], vv)
                            nc.sync.dma_start(dbg_c2_d[t, oc], c2t)
                    if dbg and b == 0 and rep == 0:
                        nc.sync.dma_start(dbg_o2_d[t], o2)
                    for oc in range(NH):
                        psp = mmpool.tile([HD, TN], f32, tag="mm",
                                          name=f"p{b}_{t}_{oc}{sfx}")
                        for k2 in range(2):
                            rhs = o2[:, k2 * 2 * TN:(k2 + 1) * 2 * TN].rearrange(
                                "p (two n) -> p two n", two=2)
                            nc.tensor.matmul(psp, wg["p"][oc][k2], rhs,
                                             start=(k2 == 0), stop=(k2 == 1),
                                             perf_mode=DR)
                        xr = xrpool.tile([HD, TN], f32, tag=f"xr{oc}",
                                         name=f"xr{oc}_b{b}_{t}{sfx}")
                        nc.sync.dma_start(
                            xr.rearrange("p (h w) -> p h w", w=W),
                            xr_d[b, oc * HD:(oc + 1) * HD, r0:r0 + TH, :])
                        ot = otpool.tile([HD, TN], f32, tag=f"ot{oc}",
                                         name=f"ot{oc}_b{b}_{t}{sfx}")
                        nc.vector.scalar_tensor_tensor(ot, psp, inv_ws, xr,
                                                       AL.mult, AL.add)
                        nc.sync.dma_start(
                            out_d[b, oc * HD:(oc + 1) * HD, r0:r0 + TH, :],
                            ot.rearrange("p (h w) -> p h w", w=W))

            stats0 = phase_qk(0)
            phase_se(0, stats0)
            if dbg and rep == 0:
                for c in range(NH):
                    nc.sync.dma_start(dbg_qk_d[c], qk[0][c])
            if BL > 1:
                stats1 = phase_qk(1)
            phase_scale_diag(0)
            phase_mconv(0)
            if dbg and rep == 0:
                for c in range(NH):
                    nc.sync.dma_start(dbg_m_d[c], mb[0][c])
            phase_out(0)
            if BL > 1:
                phase_se(1, stats1)
                phase_scale_diag(1)
                phase_mconv(1)
                phase_out(1)

        if cfg['repeat'] > 1:
            for rep in range(cfg['repeat']):
                emit_body(rep)
        else:
            emit_body(0)

    nc.compile()
    return nc


# ---------------------------------------------------------------------------
# host-side weight prep
# ---------------------------------------------------------------------------

def prep_weights(inputs, cfg):
    import ml_dtypes
    f32 = np.float32
    f8 = ml_dtypes.float8_e4m3fn
    npix = H * W

    qkv_w = np.asarray(inputs['qkv_w'], f32)
    proj_w = np.asarray(inputs['proj_w'], f32)

    def gemm_pairs(Wm):
        # Wm [512 out, 512 in] -> [oc, kc2, 128, 2, 128] fp8, prescaled
        Wt = np.ascontiguousarray(Wm.T) * WS     # [c_in, c_out]
        out = np.zeros((NH, 2, HD, 2, HD), f32)
        for oc in range(NH):
            for k2 in range(2):
                for i in range(2):
                    ci = (2 * k2 + i) * HD
                    out[oc, k2, :, i, :] = Wt[ci:ci + HD,
                                              oc * HD:(oc + 1) * HD]
        return out.astype(f8)

    sq_w = np.asarray(inputs['sq_w'], f32).reshape(DIM, 9)
    sk_w = np.asarray(inputs['sk_w'], f32).reshape(DIM, 9)
    dwc_w = np.asarray(inputs['dwc_w'], f32).reshape(DIM, 9)

    def mconv_pairs():
        out = np.zeros((NH, 9, HD, 2, HD), f32)
        idx = np.arange(HD)
        for c in range(NH):
            for j in range(9):
                out[c, j, idx, 0, idx] = sq_w[c * HD:(c + 1) * HD, j] * WS
                out[c, j, idx, 1, idx] = sk_w[c * HD:(c + 1) * HD, j] * WS
        # dram layout [head, p, tap, i, c] must match the SBUF tile dims
        return np.ascontiguousarray(out.transpose(0, 2, 1, 3, 4)).astype(f8)

    def dwc_tiles():
        pairs = np.zeros((NH, 4, HD, 2, HD), f32)
        singles = np.zeros((NH, HD, HD), f32)
        idx = np.arange(HD)
        for c in range(NH):
            for pi, (ja, jb) in enumerate(DWC_PAIRS):
                pairs[c, pi, idx, 0, idx] = dwc_w[c * HD:(c + 1) * HD, ja] * WS
                pairs[c, pi, idx, 1, idx] = dwc_w[c * HD:(c + 1) * HD, jb] * WS
            singles[c, idx, idx] = dwc_w[c * HD:(c + 1) * HD, DWC_SINGLE] * WS
        pairs = np.ascontiguousarray(pairs.transpose(0, 2, 1, 3, 4))
        return pairs.astype(f8), singles.astype(f8)

    def pool_coef(w9):
        # [DIM, 9 taps] -> [DIM, 9 stats] coefficients (includes /npix)
        coef = np.zeros((DIM, 9), f32)
        for j, (dy, dx) in enumerate(TAPS):
            wj = w9[:, j]
            coef[:, 0] += wj
            if dy == 1:
                coef[:, 1] -= wj
            if dy == -1:
                coef[:, 2] -= wj
            if dx == 1:
                coef[:, 3] -= wj
            if dx == -1:
                coef[:, 4] -= wj
            if dy and dx:
                ci = {(1, 1): 5, (1, -1): 6, (-1, 1): 7, (-1, -1): 8}[(dy, dx)]
                coef[:, ci] += wj
        return coef / npix

    coef = np.stack([pool_coef(sq_w).reshape(NH, HD, 9),
                     pool_coef(sk_w).reshape(NH, HD, 9)])

    dgw_p, dgs_p = dwc_tiles()

    x32 = np.asarray(inputs['x'], f32)
    xr = x32 + np.asarray(inputs['proj_b'], f32)[None, :, None, None]

    # SE: pooled (already /npix) -> w1 @ pooled + b1 (+ w1 @ conv_bias fold)
    def se_prep(w1, b1, w2, b2, conv_b):
        w1 = np.asarray(w1, f32)          # [NH, HD4, HD]
        b1f = (np.asarray(b1, f32) +
               np.einsum('nfc,nc->nf', w1, conv_b.reshape(NH, HD)))
        return (np.ascontiguousarray(w1.transpose(0, 2, 1)),
                b1f.reshape(NH, HD4, 1),
                np.ascontiguousarray(np.asarray(w2, f32).transpose(0, 2, 1)),
                np.asarray(b2, f32).reshape(NH, HD, 1))

    sqb = np.asarray(inputs['sq_b'], f32)
    skb = np.asarray(inputs['sk_b'], f32)
    w1q, b1q, w2q, b2q = se_prep(inputs['cq_w1'], inputs['cq_b1'],
                                 inputs['cq_w2'], inputs['cq_b2'], sqb)
    w1k, b1k, w2k, b2k = se_prep(inputs['ck_w1'], inputs['ck_b1'],
                                 inputs['ck_w2'], inputs['ck_b2'], skb)

    return dict(
        w_q=gemm_pairs(qkv_w[0:DIM]),
        w_k=gemm_pairs(qkv_w[DIM:2 * DIM]),
        w_v=gemm_pairs(qkv_w[2 * DIM:3 * DIM]),
        w_p=gemm_pairs(proj_w),
        dgm=mconv_pairs(), dgw=dgw_p, dgs=dgs_p,
        coef=np.ascontiguousarray(coef),
        sq_b=sqb.reshape(DIM, 1), sk_b=skb.reshape(DIM, 1),
        dwc_b=np.asarray(inputs['dwc_b'], f32).reshape(DIM, 1),
        se_w1q=w1q, se_b1q=b1q, se_w2q=w2q, se_b2q=b2q,
        se_w1k=w1k, se_b1k=b1k, se_w2k=w2k, se_b2k=b2k,
        _x8=x32.astype(f8), _xr=xr,
    )


_CACHE = {}


def _get_compiled(cfg_key, cfg):
    if cfg_key not in _CACHE:
        _CACHE[cfg_key] = build_nc(cfg)
    return _CACHE[cfg_key]


def kernel(**inputs):
    from concourse import bass_utils
    cfg = default_cfg()
    nc = _get_compiled('main', cfg)
    w = prep_weights(inputs, cfg)
    x8 = w.pop('_x8')
    xr = w.pop('_xr')
    BL = cfg['b_local']
    in_maps = []
    for core in range(N_CORES):
        m = dict(w)
        m['x8'] = np.ascontiguousarray(x8[core * BL:(core + 1) * BL])
        m['xr'] = np.ascontiguousarray(xr[core * BL:(core + 1) * BL])
        in_maps.append(m)
    res = bass_utils.run_bass_kernel_spmd(nc, in_maps, core_ids=list(range(N_CORES)))
    out = np.empty((B, DIM, H_FULL, W), np.float32)
    for core in range(N_CORES):
        out[core * BL:(core + 1) * BL] = res.results[core]['out']
    return out


# revision 17
# speedup vs baseline: 1.0163x; 1.0163x over previous
"""Trainium2 Bass kernel for nn_CASAtt_MultiHead_v1 (CAS attention block).

Reference computation (per sample):
    qkv = 1x1 conv (qkv_w) -> q, k, v                        [512, 56, 56] each
    q <- SE(dwconv3x3(q, sq_w, sq_b))   (per-head squeeze-excite)
    k <- SE(dwconv3x3(k, sk_w, sk_b))
    out = proj(dwconv3x3(q + k, dwc_w, dwc_b) * v) + proj_b + x

Distribution: data-parallel over batch, 2 samples per NeuronCore x 8 cores.

Strategy (all PE matmuls in fp8e4m3 with DoubleRow pairing, f32 PSUM):
  * GEMMs: contraction K=512 done as 2 DoubleRow matmuls (K=256 each,
    channel-block pairs) per 128-out block -- 2x fewer PE column-cycles
    than bf16.
  * The SE scales s_q, s_k are computed WITHOUT materializing conv1:
    with zero padding, mean(dwconv3x3(q)) is a per-channel linear
    combination of 9 cheap stats (total sum via drain accum_out, 2 edge
    row sums, 2 edge col sums, 4 corners).  Host precomputes the [C, 9]
    coefficient table.
  * m = s_q*conv_sq(q) + s_k*conv_sk(k) is then computed directly as 9
    cross-branch DoubleRow pair matmuls: tap j of q paired with tap j of
    k (q/k live in one SBUF tile, pair stride = buffer pitch), with the
    diagonal tap weights scaled by s at runtime on DVE.  This fuses both
    conv1 branches AND the SE-scale merge into one accumulation.
  * dwc conv on m: 4 DoubleRow tap-pair matmuls + 1 single matmul.
  * fp8 weights are prescaled by WS=32 (values ~0.05 would be subnormal
    in e4m3); drains compensate with ACT scale=1/WS.
  * Residual: host ships xr = x + proj_b in f32; the proj drain is a
    single DVE STT (psum * 1/WS + xr).
Conv matmuls operate on zero-padded HPxWP buffers (WP=60) over full
padded rows; garbage in pad columns is never read.
"""

import numpy as np

DIM = 512
NH = 4
HD = 128
HD4 = 32
B, H_FULL, W = 16, 56, 56
N_CORES = 8
WS = 32.0

H = H_FULL
WP = W + 4  # 60: even row pitch (4B-aligned fp8 rows for 2x ACT/DVE modes)
HP = H + 2
PADN = HP * WP          # 3480
PITCH = PADN + 4        # per-branch pitch (4B multiple for int32 memset)
TAPS = [(dy, dx) for dy in (-1, 0, 1) for dx in (-1, 0, 1)]
DWC_PAIRS = [(0, 1), (2, 3), (4, 5), (6, 7)]
DWC_SINGLE = 8


def default_cfg():
    return dict(
        b_local=B // N_CORES,
        rows_per_tile=8,
        repeat=1,
    )


def build_nc(cfg):
    """Build + compile the Bacc program for one core (SPMD across 8)."""
    import concourse.bass as bass
    import concourse.mybir as mybir
    import concourse.tile as tile
    import bass_rust
    from concourse import bacc
    from contextlib import ExitStack

    f32 = mybir.dt.float32
    bf16 = mybir.dt.bfloat16
    fp8 = mybir.dt.float8e4
    DR = mybir.MatmulPerfMode.DoubleRow

    BL = cfg['b_local']
    TH = cfg['rows_per_tile']
    NT = H // TH
    assert NT * TH == H
    TN = TH * W
    TPAD = TH * WP
    AF = mybir.ActivationFunctionType
    AL = mybir.AluOpType

    nc = bacc.Bacc("TRN2", target_bir_lowering=False, debug=False,
                   enable_asserts=False, num_devices=N_CORES)

    # ---------------- DRAM I/O ----------------
    x_d = nc.dram_tensor("x8", [BL, DIM, H, W], fp8, kind="ExternalInput").ap()
    xr_d = nc.dram_tensor("xr", [BL, DIM, H, W], f32, kind="ExternalInput").ap()
    out_d = nc.dram_tensor("out", [BL, DIM, H, W], f32, kind="ExternalOutput").ap()
    # GEMM weights: [gemm][oc][kc2][128, 2, 128] fp8 (prescaled by WS)
    wg_d = {g: nc.dram_tensor(f"w_{g}", [NH, 2, HD, 2, HD], fp8,
                              kind="ExternalInput").ap()
            for g in ("q", "k", "v", "p")}
    # m-conv diag pairs: [head][tap][128, 2, 128] fp8 (prescaled by WS)
    dgm_d = nc.dram_tensor("dgm", [NH, HD, 9, 2, HD], fp8,
                           kind="ExternalInput").ap()
    # dwc diag: pairs [head][4][128, 2, 128] + singles [head][128, 128]
    dgw_d = nc.dram_tensor("dgw", [NH, HD, 4, 2, HD], fp8,
                           kind="ExternalInput").ap()
    dgs_d = nc.dram_tensor("dgs", [NH, HD, HD], fp8,
                           kind="ExternalInput").ap()
    # pooling coef [2 branches][head][128, 9] f32 (includes /npix)
    coef_d = nc.dram_tensor("coef", [2, NH, HD, 9], f32,
                            kind="ExternalInput").ap()
    # biases
    b1_d = [nc.dram_tensor(n, [DIM, 1], f32, kind="ExternalInput").ap()
            for n in ("sq_b", "sk_b")]
    dwcb_d = nc.dram_tensor("dwc_b", [DIM, 1], f32, kind="ExternalInput").ap()
    # SE weights (f32); seb1 already includes w1 @ conv-bias fold
    sew1_d = [nc.dram_tensor(n, [NH, HD, HD4], f32, kind="ExternalInput").ap()
              for n in ("se_w1q", "se_w1k")]
    seb1_d = [nc.dram_tensor(n, [NH, HD4, 1], f32, kind="ExternalInput").ap()
              for n in ("se_b1q", "se_b1k")]
    sew2_d = [nc.dram_tensor(n, [NH, HD4, HD], f32, kind="ExternalInput").ap()
              for n in ("se_w2q", "se_w2k")]
    seb2_d = [nc.dram_tensor(n, [NH, HD, 1], f32, kind="ExternalInput").ap()
              for n in ("se_b2q", "se_b2k")]

    dbg = cfg.get('debug', False)
    if dbg:
        dbg_qk_d = nc.dram_tensor("dbg_qk", [NH, HD, 2 * PITCH], fp8,
                                  kind="ExternalOutput").ap()
        dbg_m_d = nc.dram_tensor("dbg_m", [NH, HD, PITCH], fp8,
                                 kind="ExternalOutput").ap()
        dbg_s_d = nc.dram_tensor("dbg_s", [2, NH, HD, 1], f32,
                                 kind="ExternalOutput").ap()
        dbg_st_d = nc.dram_tensor("dbg_st", [2, NH, HD, 9], f32,
                                  kind="ExternalOutput").ap()
        dbg_pool_d = nc.dram_tensor("dbg_pool", [2, NH, HD, 1], f32,
                                    kind="ExternalOutput").ap()
        dbg_o2_d = nc.dram_tensor("dbg_o2", [H // cfg['rows_per_tile'],
                                             HD, 4 * cfg['rows_per_tile'] * W],
                                  fp8, kind="ExternalOutput").ap()
        dbg_v_d = nc.dram_tensor("dbg_v", [H // cfg['rows_per_tile'], NH,
                                           HD, cfg['rows_per_tile'] * W],
                                 mybir.dt.bfloat16, kind="ExternalOutput").ap()
        dbg_c2_d = nc.dram_tensor("dbg_c2", [H // cfg['rows_per_tile'], NH,
                                             HD, cfg['rows_per_tile'] * W],
                                  mybir.dt.bfloat16, kind="ExternalOutput").ap()

    with tile.TileContext(nc) as tc, ExitStack() as ctx:
        const = ctx.enter_context(tc.tile_pool(name="const", bufs=1))
        big = ctx.enter_context(tc.tile_pool(name="big", bufs=1))
        sdg = ctx.enter_context(tc.tile_pool(name="sdg", bufs=2))
        xpool = ctx.enter_context(tc.tile_pool(name="xpool", bufs=2))
        vpool = ctx.enter_context(tc.tile_pool(name="vpool", bufs=2))
        o2pool = ctx.enter_context(tc.tile_pool(name="o2pool", bufs=2))
        otpool = ctx.enter_context(tc.tile_pool(name="otpool", bufs=2))
        xrpool = ctx.enter_context(tc.tile_pool(name="xrpool", bufs=2))
        statpool = ctx.enter_context(tc.tile_pool(name="statpool", bufs=2))
        mmpool = ctx.enter_context(tc.tile_pool(name="mmpool", bufs=6, space="PSUM"))
        sepool = ctx.enter_context(tc.tile_pool(name="sepool", bufs=2, space="PSUM"))

        # ---------- persistent SBUF ----------
        # q/k padded pair buffers (per sample): q at [1, 1+PADN), k at
        # [PITCH+1, ...).  Per-sample buffers so sample b+1's GEMM drains
        # can overlap sample b's m-conv.
        qk = [[big.tile([HD, 2 * PITCH], fp8, name=f"qk{b}_{c}")
               for c in range(NH)] for b in range(BL)]
        mb = [[big.tile([HD, PITCH], fp8, name=f"m{b}_{c}")
               for c in range(NH)] for b in range(BL)]
        # zero pads via int32 bitcast (4x fewer elems); qk on DVE and m
        # on ACT so the two halves of the startup zeroing run concurrently
        for row in qk:
            for t in row:
                nc.vector.memset(t.bitcast(mybir.dt.int32), 0)
        for row in mb:
            for t in row:
                nc.scalar.memzero(t.bitcast(mybir.dt.int32))

        def pad3(t, half):
            return t[:, half * PITCH + 1:half * PITCH + 1 + PADN].rearrange(
                "p (h w) -> p h w", w=WP)

        qk3 = [[[pad3(t, half) for half in range(2)] for t in qk[b]]
               for b in range(BL)]
        m3 = [[pad3(t, 0) for t in mb[b]] for b in range(BL)]

        # ---------- static weights ----------
        wg = {}
        for g in ("q", "k", "v", "p"):
            wg[g] = [[const.tile([HD, 2, HD], fp8, name=f"w{g}_{oc}_{k2}")
                      for k2 in range(2)] for oc in range(NH)]
            for oc in range(NH):
                for k2 in range(2):
                    nc.sync.dma_start(wg[g][oc][k2], wg_d[g][oc, k2])
        # raw m-conv diag pairs, one tile per head [128, 9, 2, 128]
        dgm = [const.tile([HD, 9, 2, HD], fp8, name=f"dgm{c}") for c in range(NH)]
        for c in range(NH):
            nc.sync.dma_start(dgm[c], dgm_d[c])
        dgw = [const.tile([HD, 4, 2, HD], fp8, name=f"dgw{c}") for c in range(NH)]
        dgs = [const.tile([HD, HD], fp8, name=f"dgs{c}") for c in range(NH)]
        for c in range(NH):
            nc.sync.dma_start(dgw[c], dgw_d[c])
            nc.sync.dma_start(dgs[c], dgs_d[c])
        coef = [[const.tile([HD, 9], f32, name=f"coef{br}_{c}") for c in range(NH)]
                for br in range(2)]
        for br in range(2):
            for c in range(NH):
                nc.sync.dma_start(coef[br][c], coef_d[br, c])
        bias1 = [[const.tile([HD, 1], f32, name=f"b1_{br}_{c}") for c in range(NH)]
                 for br in range(2)]
        dwcb = [const.tile([HD, 1], f32, name=f"dwcb{c}") for c in range(NH)]
        inv_ws = const.tile([HD, 1], f32, name="inv_ws")
        nc.vector.memset(inv_ws, 1.0 / WS)
        for c in range(NH):
            sl = slice(c * HD, (c + 1) * HD)
            for br in range(2):
                nc.sync.dma_start(bias1[br][c], b1_d[br][sl])
            nc.sync.dma_start(dwcb[c], dwcb_d[sl])
        sew1 = [[const.tile([HD, HD4], f32, name=f"sew1_{br}_{c}") for c in range(NH)]
                for br in range(2)]
        seb1 = [[const.tile([HD4, 1], f32, name=f"seb1_{br}_{c}") for c in range(NH)]
                for br in range(2)]
        sew2 = [[const.tile([HD4, HD], f32, name=f"sew2_{br}_{c}") for c in range(NH)]
                for br in range(2)]
        seb2 = [[const.tile([HD, 1], f32, name=f"seb2_{br}_{c}") for c in range(NH)]
                for br in range(2)]
        for br in range(2):
            for c in range(NH):
                nc.sync.dma_start(sew1[br][c], sew1_d[br][c])
                nc.sync.dma_start(seb1[br][c], seb1_d[br][c])
                nc.sync.dma_start(sew2[br][c], sew2_d[br][c])
                nc.sync.dma_start(seb2[br][c], seb2_d[br][c])

        def pair_rhs(tbuf, r0, j, pitch):
            """[128, 2, TPAD] AP: tap j over padded rows r0+1..r0+TH of both
            halves of a pair buffer (pair stride = pitch)."""
            dy, dx = TAPS[j]
            base = 1 + (r0 + 1) * WP + dy * WP + dx
            return bass_rust.AP(
                tensor=tbuf.tensor, offset=tbuf.offset + base,
                ap=[[tbuf.shape[1], HD], [pitch, 2], [1, TPAD]])

        def tap_rhs(tbuf, r0, ja, jb):
            """[128, 2, TPAD] AP on a single-pitch buffer: taps ja, jb."""
            dya, dxa = TAPS[ja]
            dyb, dxb = TAPS[jb]
            da = dya * WP + dxa
            db = dyb * WP + dxb
            base = 1 + (r0 + 1) * WP + da
            return bass_rust.AP(
                tensor=tbuf.tensor, offset=tbuf.offset + base,
                ap=[[tbuf.shape[1], HD], [db - da, 2], [1, TPAD]])

        def single_rhs(tbuf, r0, j):
            dy, dx = TAPS[j]
            base = 1 + (r0 + 1) * WP + dy * WP + dx
            return tbuf[:, base:base + TPAD]

        def emit_body(rep):
            sfx = f"_r{rep}" if cfg['repeat'] > 1 else ""
            s_scale = [[[None] * NH for _ in range(2)] for _ in range(BL)]
            sdiag = [None] * BL

            def phase_qk(b):
                # q,k GEMMs -> qk pair buffers (fp8), with accum_out stats
                stats_s = [[statpool.tile([HD, NT], f32, tag=f"ss{br}_{oc}",
                                          name=f"ss{b}_{br}_{oc}{sfx}")
                            for oc in range(NH)] for br in range(2)]
                for t in range(NT):
                    r0 = t * TH
                    xt = xpool.tile([HD, 4 * TN], fp8, tag="xt",
                                    name=f"xt_b{b}_{t}{sfx}")
                    for kc in range(NH):
                        nc.sync.dma_start(
                            xt[:, kc * TN:(kc + 1) * TN].rearrange(
                                "p (h w) -> p h w", w=W),
                            x_d[b, kc * HD:(kc + 1) * HD, r0:r0 + TH, :])
                    for br, g in enumerate(("q", "k")):
                        for oc in range(NH):
                            ps = mmpool.tile([HD, TN], f32, tag="mm",
                                             name=f"g{b}_{g}_{t}_{oc}{sfx}")
                            for k2 in range(2):
                                rhs = xt[:, k2 * 2 * TN:(k2 + 1) * 2 * TN].rearrange(
                                    "p (two n) -> p two n", two=2)
                                nc.tensor.matmul(ps, wg[g][oc][k2], rhs,
                                                 start=(k2 == 0), stop=(k2 == 1),
                                                 perf_mode=DR)
                            nc.scalar.activation(
                                qk3[b][oc][br][:, 1 + r0:1 + r0 + TH, 1:1 + W],
                                ps.rearrange("p (h w) -> p h w", w=W),
                                AF.Identity, scale=inv_ws,
                                accum_out=stats_s[br][oc][:, t:t + 1])
                return stats_s

            def phase_se(b, stats_s):
                # boundary-corrected pooling + SE -> s_scale
                for br in range(2):
                    for oc in range(NH):
                        buf = qk[b][oc]
                        # flat offset of interior pixel (0, 0): slop(1) + row
                        # pad(WP) + col pad(1)
                        off = buf.offset + br * PITCH + 1
                        st = statpool.tile([HD, 9], f32, tag="st9",
                                           name=f"st9_{b}_{br}_{oc}{sfx}")
                        # S
                        nc.vector.tensor_reduce(st[:, 0:1], stats_s[br][oc],
                                                mybir.AxisListType.X, AL.add)
                        # rows 0 and H-1 (padded rows 1, H): offsets 61, H*WP+1
                        rows = bass_rust.AP(
                            tensor=buf.tensor, offset=off + WP + 1,
                            ap=[[buf.shape[1], HD], [(H - 1) * WP, 2], [1, W]])
                        nc.vector.tensor_reduce(st[:, 1:3], rows,
                                                mybir.AxisListType.X, AL.add)
                        # cols 0 and W-1: offsets 61, 61+W-1; stride WP over H
                        cols = bass_rust.AP(
                            tensor=buf.tensor, offset=off + WP + 1,
                            ap=[[buf.shape[1], HD], [W - 1, 2], [WP, H]])
                        nc.vector.tensor_reduce(st[:, 3:5], cols,
                                                mybir.AxisListType.X, AL.add)
                        # 4 corners
                        corners = bass_rust.AP(
                            tensor=buf.tensor, offset=off + WP + 1,
                            ap=[[buf.shape[1], HD], [(H - 1) * WP, 2], [W - 1, 2]])
                        nc.vector.tensor_copy(
                            st[:, 5:9].rearrange("p (a b) -> p a b", a=2), corners)
                        # pooled = sum(st * coef)
                        prod = statpool.tile([HD, 9], f32, tag="pr9",
                                             name=f"pr9_{b}_{br}_{oc}{sfx}")
                        nc.vector.tensor_tensor(prod, st, coef[br][oc], AL.mult)
                        pooled = const.tile([HD, 1], f32, tag="pooled", bufs=4,
                                            name=f"pool{b}_{br}_{oc}{sfx}")
                        nc.vector.tensor_reduce(pooled, prod,
                                                mybir.AxisListType.X, AL.add)
                        ps1 = sepool.tile([HD4, 1], f32, tag="se",
                                          name=f"se1_{b}_{br}_{oc}{sfx}")
                        nc.tensor.matmul(ps1, sew1[br][oc], pooled,
                                         start=True, stop=True)
                        hvec = const.tile([HD4, 1], f32, tag="hvec", bufs=4,
                                          name=f"h{b}_{br}_{oc}{sfx}")
                        nc.scalar.activation(hvec, ps1, AF.Relu,
                                             bias=seb1[br][oc])
                        ps2 = sepool.tile([HD, 1], f32, tag="se",
                                          name=f"se2_{b}_{br}_{oc}{sfx}")
                        nc.tensor.matmul(ps2, sew2[br][oc], hvec,
                                         start=True, stop=True)
                        s_sb = const.tile([HD, 1], f32, tag="s_scale", bufs=16,
                                          name=f"s{b}_{br}_{oc}{sfx}")
                        nc.scalar.activation(s_sb, ps2, AF.Sigmoid,
                                             bias=seb2[br][oc])
                        s_scale[b][br][oc] = s_sb
                        if dbg and b == 0 and rep == 0:
                            nc.sync.dma_start(dbg_st_d[br, oc], st)
                            nc.sync.dma_start(dbg_pool_d[br, oc], pooled)
                            nc.sync.dma_start(dbg_s_d[br, oc], s_sb)

            def phase_scale_diag(b):
                # scaled diag pairs: sd[:, j, 0, :] = s_q * dgm[:, j, 0, :]
                sd = [sdg.tile([HD, 9, 2, HD], fp8, tag=f"sd{c}",
                               name=f"sd{c}_b{b}{sfx}") for c in range(NH)]
                for c in range(NH):
                    for br in range(2):
                        nc.vector.tensor_scalar(
                            sd[c][:, :, br, :], dgm[c][:, :, br, :],
                            s_scale[b][br][c], None, AL.mult)
                sdiag[b] = sd

            def phase_mconv(b):
                # m = s_q*conv_sq(q) + s_k*conv_sk(k) : 9 DR pair MMs/tile
                for oc in range(NH):
                    # mbias = s_q*sq_b + s_k*sk_b
                    mb_t = const.tile([HD, 1], f32, tag="mbias", bufs=4,
                                      name=f"mb{b}_{oc}{sfx}")
                    nc.vector.tensor_scalar(mb_t, bias1[1][oc],
                                            s_scale[b][1][oc], None, AL.mult)
                    nc.vector.scalar_tensor_tensor(
                        mb_t, bias1[0][oc], s_scale[b][0][oc], mb_t,
                        AL.mult, AL.add)
                    for t in range(NT):
                        r0 = t * TH
                        ps = mmpool.tile([HD, TPAD], f32, tag="mm",
                                         name=f"mc{b}_{t}_{oc}{sfx}")
                        for j in range(9):
                            nc.tensor.matmul(ps, sdiag[b][oc][:, j],
                                             pair_rhs(qk[b][oc], r0, j, PITCH),
                                             start=(j == 0), stop=(j == 8),
                                             perf_mode=DR)
                        nc.scalar.activation(
                            m3[b][oc][:, 1 + r0:1 + r0 + TH, 1:1 + W],
                            ps.rearrange("p (h w) -> p h w",
                                         w=WP)[:, :, 1:1 + W],
                            AF.Identity, bias=mb_t, scale=inv_ws)

            def phase_out(b):
                # dwc conv + v GEMM + o2 + proj + residual
                for t in range(NT):
                    r0 = t * TH
                    xt = xpool.tile([HD, 4 * TN], fp8, tag="xt",
                                    name=f"x2_b{b}_{t}{sfx}")
                    for kc in range(NH):
                        nc.sync.dma_start(
                            xt[:, kc * TN:(kc + 1) * TN].rearrange(
                                "p (h w) -> p h w", w=W),
                            x_d[b, kc * HD:(kc + 1) * HD, r0:r0 + TH, :])
                    o2 = o2pool.tile([HD, 4 * TN], fp8, tag="o2",
                                     name=f"o2_b{b}_{t}{sfx}")
                    for oc in range(NH):
                        # v = Wv x
                        psv = mmpool.tile([HD, TN], f32, tag="mm",
                                          name=f"v{b}_{t}_{oc}{sfx}")
                        for k2 in range(2):
                            rhs = xt[:, k2 * 2 * TN:(k2 + 1) * 2 * TN].rearrange(
                                "p (two n) -> p two n", two=2)
                            nc.tensor.matmul(psv, wg["v"][oc][k2], rhs,
                                             start=(k2 == 0), stop=(k2 == 1),
                                             perf_mode=DR)
                        vv = vpool.tile([HD, TN], bf16, tag=f"vt{oc}",
                                        name=f"vt{oc}_b{b}_{t}{sfx}")
                        nc.vector.tensor_scalar(vv, psv, 1.0 / WS, None, AL.mult)
                        # c2 = dwc(m) + dwc_b
                        psc = mmpool.tile([HD, TPAD], f32, tag="mm",
                                          name=f"c2{b}_{t}_{oc}{sfx}")
                        for pi, (ja, jb) in enumerate(DWC_PAIRS):
                            nc.tensor.matmul(psc, dgw[oc][:, pi],
                                             tap_rhs(mb[b][oc], r0, ja, jb),
                                             start=(pi == 0), stop=False,
                                             perf_mode=DR)
                        nc.tensor.matmul(psc, dgs[oc],
                                         single_rhs(mb[b][oc], r0, DWC_SINGLE),
                                         start=False, stop=True)
                        c2t = o2pool.tile([HD, TN], bf16, tag="c2t", bufs=3,
                                          name=f"c2t_{oc}_b{b}_{t}{sfx}")
                        nc.vector.tensor_scalar(
                            c2t.rearrange("p (h w) -> p h w", w=W),
                            psc.rearrange("p (h w) -> p h w", w=WP)[:, :, 1:1 + W],
                            1.0 / WS, dwcb[oc], AL.mult, AL.add)
                        # o2 = c2 * v  (bf16 in, fp8 out)
                        nc.vector.tensor_tensor(
                            o2[:, oc * TN:(oc + 1) * TN], c2t, vv, AL.mult)
                        if dbg and b == 0 and rep == 0:
                            nc.sync.dma_start(dbg_v_d[t, oc], vv)
                            nc.sync.dma_start(dbg_c2_d[t, oc], c2t)
                    if dbg and b == 0 and rep == 0:
                        nc.sync.dma_start(dbg_o2_d[t], o2)
                    for oc in range(NH):
                        psp = mmpool.tile([HD, TN], f32, tag="mm",
                                          name=f"p{b}_{t}_{oc}{sfx}")
                        for k2 in range(2):
                            rhs = o2[:, k2 * 2 * TN:(k2 + 1) * 2 * TN].rearrange(
                                "p (two n) -> p two n", two=2)
                            nc.tensor.matmul(psp, wg["p"][oc][k2], rhs,
                                             start=(k2 == 0), stop=(k2 == 1),
                                             perf_mode=DR)
                        xr = xrpool.tile([HD, TN], f32, tag=f"xr{oc}",
                                         name=f"xr{oc}_b{b}_{t}{sfx}")
                        nc.sync.dma_start(
                            xr.rearrange("p (h w) -> p h w", w=W),
                            xr_d[b, oc * HD:(oc + 1) * HD, r0:r0 + TH, :])
                        ot = otpool.tile([HD, TN], f32, tag=f"ot{oc}",
                                         name=f"ot{oc}_b{b}_{t}{sfx}")
                        nc.vector.scalar_tensor_tensor(ot, psp, inv_ws, xr,
                                                       AL.mult, AL.add)
                        nc.sync.dma_start(
                            out_d[b, oc * HD:(oc + 1) * HD, r0:r0 + TH, :],
                            ot.rearrange("p (h w) -> p h w", w=W))

            stats0 = phase_qk(0)
            phase_se(0, stats0)
            if dbg and rep == 0:
                for c in range(NH):
                    nc.sync.dma_start(dbg_qk_d[c], qk[0][c])
            if BL > 1:
                stats1 = phase_qk(1)
            phase_scale_diag(0)
            phase_mconv(0)
            if dbg and rep == 0:
                for c in range(NH):
                    nc.sync.dma_start(dbg_m_d[c], mb[0][c])
            phase_out(0)
            if BL > 1:
                phase_se(1, stats1)
                phase_scale_diag(1)
                phase_mconv(1)
                phase_out(1)

        if cfg['repeat'] > 1:
            for rep in range(cfg['repeat']):
                emit_body(rep)
        else:
            emit_body(0)

    nc.compile()
    return nc


# ---------------------------------------------------------------------------
# host-side weight prep
# ---------------------------------------------------------------------------

def prep_weights(inputs, cfg):
    import ml_dtypes
    f32 = np.float32
    f8 = ml_dtypes.float8_e4m3fn
    npix = H * W

    qkv_w = np.asarray(inputs['qkv_w'], f32)
    proj_w = np.asarray(inputs['proj_w'], f32)

    def gemm_pairs(Wm):
        # Wm [512 out, 512 in] -> [oc, kc2, 128, 2, 128] fp8, prescaled
        Wt = np.ascontiguousarray(Wm.T) * WS     # [c_in, c_out]
        out = np.zeros((NH, 2, HD, 2, HD), f32)
        for oc in range(NH):
            for k2 in range(2):
                for i in range(2):
                    ci = (2 * k2 + i) * HD
                    out[oc, k2, :, i, :] = Wt[ci:ci + HD,
                                              oc * HD:(oc + 1) * HD]
        return out.astype(f8)

    sq_w = np.asarray(inputs['sq_w'], f32).reshape(DIM, 9)
    sk_w = np.asarray(inputs['sk_w'], f32).reshape(DIM, 9)
    dwc_w = np.asarray(inputs['dwc_w'], f32).reshape(DIM, 9)

    def mconv_pairs():
        out = np.zeros((NH, 9, HD, 2, HD), f32)
        idx = np.arange(HD)
        for c in range(NH):
            for j in range(9):
                out[c, j, idx, 0, idx] = sq_w[c * HD:(c + 1) * HD, j] * WS
                out[c, j, idx, 1, idx] = sk_w[c * HD:(c + 1) * HD, j] * WS
        # dram layout [head, p, tap, i, c] must match the SBUF tile dims
        return np.ascontiguousarray(out.transpose(0, 2, 1, 3, 4)).astype(f8)

    def dwc_tiles():
        pairs = np.zeros((NH, 4, HD, 2, HD), f32)
        singles = np.zeros((NH, HD, HD), f32)
        idx = np.arange(HD)
        for c in range(NH):
            for pi, (ja, jb) in enumerate(DWC_PAIRS):
                pairs[c, pi, idx, 0, idx] = dwc_w[c * HD:(c + 1) * HD, ja] * WS
                pairs[c, pi, idx, 1, idx] = dwc_w[c * HD:(c + 1) * HD, jb] * WS
            singles[c, idx, idx] = dwc_w[c * HD:(c + 1) * HD, DWC_SINGLE] * WS
        pairs = np.ascontiguousarray(pairs.transpose(0, 2, 1, 3, 4))
        return pairs.astype(f8), singles.astype(f8)

    def pool_coef(w9):
        # [DIM, 9 taps] -> [DIM, 9 stats] coefficients (includes /npix)
        coef = np.zeros((DIM, 9), f32)
        for j, (dy, dx) in enumerate(TAPS):
            wj = w9[:, j]
            coef[:, 0] += wj
            if dy == 1:
                coef[:, 1] -= wj
            if dy == -1:
                coef[:, 2] -= wj
            if dx == 1:
                coef[:, 3] -= wj
            if dx == -1:
                coef[:, 4] -= wj
            if dy and dx:
                ci = {(1, 1): 5, (1, -1): 6, (-1, 1): 7, (-1, -1): 8}[(dy, dx)]
                coef[:, ci] += wj
        return coef / npix

    coef = np.stack([pool_coef(sq_w).reshape(NH, HD, 9),
                     pool_coef(sk_w).reshape(NH, HD, 9)])

    dgw_p, dgs_p = dwc_tiles()

    x32 = np.asarray(inputs['x'], f32)
    xr = x32 + np.asarray(inputs['proj_b'], f32)[None, :, None, None]

    # SE: pooled (already /npix) -> w1 @ pooled + b1 (+ w1 @ conv_bias fold)
    def se_prep(w1, b1, w2, b2, conv_b):
        w1 = np.asarray(w1, f32)          # [NH, HD4, HD]
        b1f = (np.asarray(b1, f32) +
               np.einsum('nfc,nc->nf', w1, conv_b.reshape(NH, HD)))
        return (np.ascontiguousarray(w1.transpose(0, 2, 1)),
                b1f.reshape(NH, HD4, 1),
                np.ascontiguousarray(np.asarray(w2, f32).transpose(0, 2, 1)),
                np.asarray(b2, f32).reshape(NH, HD, 1))

    sqb = np.asarray(inputs['sq_b'], f32)
    skb = np.asarray(inputs['sk_b'], f32)
    w1q, b1q, w2q, b2q = se_prep(inputs['cq_w1'], inputs['cq_b1'],
                                 inputs['cq_w2'], inputs['cq_b2'], sqb)
    w1k, b1k, w2k, b2k = se_prep(inputs['ck_w1'], inputs['ck_b1'],
                                 inputs['ck_w2'], inputs['ck_b2'], skb)

    return dict(
        w_q=gemm_pairs(qkv_w[0:DIM]),
        w_k=gemm_pairs(qkv_w[DIM:2 * DIM]),
        w_v=gemm_pairs(qkv_w[2 * DIM:3 * DIM]),
        w_p=gemm_pairs(proj_w),
        dgm=mconv_pairs(), dgw=dgw_p, dgs=dgs_p,
        coef=np.ascontiguousarray(coef),
        sq_b=sqb.reshape(DIM, 1), sk_b=skb.reshape(DIM, 1),
        dwc_b=np.asarray(inputs['dwc_b'], f32).reshape(DIM, 1),
        se_w1q=w1q, se_b1q=b1q, se_w2q=w2q, se_b2q=b2q,
        se_w1k=w1k, se_b1k=b1k, se_w2k=w2k, se_b2k=b2k,
        _x8=x32.astype(f8), _xr=xr,
    )


_CACHE = {}


def _get_compiled(cfg_key, cfg):
    if cfg_key not in _CACHE:
        _CACHE[cfg_key] = build_nc(cfg)
    return _CACHE[cfg_key]


def kernel(**inputs):
    from concourse import bass_utils
    cfg = default_cfg()
    nc = _get_compiled('main', cfg)
    w = prep_weights(inputs, cfg)
    x8 = w.pop('_x8')
    xr = w.pop('_xr')
    BL = cfg['b_local']
    in_maps = []
    for core in range(N_CORES):
        m = dict(w)
        m['x8'] = np.ascontiguousarray(x8[core * BL:(core + 1) * BL])
        m['xr'] = np.ascontiguousarray(xr[core * BL:(core + 1) * BL])
        in_maps.append(m)
    res = bass_utils.run_bass_kernel_spmd(nc, in_maps, core_ids=list(range(N_CORES)))
    out = np.empty((B, DIM, H_FULL, W), np.float32)
    for core in range(N_CORES):
        out[core * BL:(core + 1) * BL] = res.results[core]['out']
    return out
